# revision 20
# baseline (speedup 1.0000x reference)
"""Chamfer-distance loss (CCHLoss) kernel for 8 Trainium2 NeuronCores.

Contract: kernel(**inputs) takes the FULL unsharded inputs
  v:        (16, 2048, 3) f32
  v_pred:   (16, 2048, 3) f32
  mask:     (4, 4, 2, 32, 32) f32
  pred_dw:  (16, 2048, 3) f32
and returns (loss, loss_normals) matching reference().

Strategy: data-parallel over the B=16 batch dim, 2 batches per core.
Per batch the 2048x2048 squared-distance matrix is produced by TensorE
via a K=13 matmul (bf16 hi/lo split of both the -2x*y products and the
|x|^2 / |y|^2 norm rows, all packed on HOST) in 16 PSUM groups of
[128, 2048].  ScalarE evacuates each group to SBUF bf16; VectorE
reduces it with tensor_tensor min trees:
  - per group a fold chain 2048->128 plus one tensor_reduce gives the
    row-min (-> cham_pred),
  - a bf16 min pyramid accumulates the column-min (-> colacc),
  - PE transposes + reduce fold colacc's 128 partitions (-> cham_v),
  - mask-weighted sums reduce everything to per-core scalars.
Host only shards/packs inputs and sums 8 cores' partial sums.
"""

import numpy as np

B, P1, P2, D = 16, 2048, 2048, 3
NCORES = 8
BPC = B // NCORES  # batches per core
NT = P1 // 128     # i-tiles per batch
NJ = P2 // 512     # matmul j-chunks per group
NC128 = P2 // 128  # 128-wide j-chunks (transpose fold)

_CACHE = {}


def build_bass():
    """Build + compile the per-core Bass program (same program all 8 cores)."""
    import concourse.bacc as bacc
    import concourse.tile as tile
    from concourse import mybir
    from concourse.masks import make_identity

    f32 = mybir.dt.float32
    bf16 = mybir.dt.bfloat16
    Alu = mybir.AluOpType
    Act = mybir.ActivationFunctionType
    X = mybir.AxisListType.X

    nc = bacc.Bacc("TRN2", target_bir_lowering=False, debug=False)

    xprod_h = nc.dram_tensor("xprod", (BPC, 13, P1), bf16, kind="ExternalInput")
    yprod_h = nc.dram_tensor("yprod", (BPC, 13, P2), bf16, kind="ExternalInput")
    maskT_h = nc.dram_tensor("maskT", (BPC, 128, NC128), f32, kind="ExternalInput")
    dw_h = nc.dram_tensor("dw", (128, BPC * 48), f32, kind="ExternalInput")
    out_h = nc.dram_tensor("out", (1, 8), f32, kind="ExternalOutput")

    with tile.TileContext(nc) as tc:
        with (
            tc.tile_pool(name="consts", bufs=1) as consts,
            tc.tile_pool(name="opnds", bufs=2) as opnds,
            tc.tile_pool(name="scr", bufs=3) as scr,
            tc.tile_pool(name="small", bufs=4) as small,
            tc.tile_pool(name="ps", bufs=2, space="PSUM") as ps,
        ):
            ident = consts.tile([128, 128], bf16)
            make_identity(nc, ident)
            ones128 = consts.tile([128, 1], f32)
            nc.vector.memset(ones128, 1.0)
            partials = consts.tile([128, 8], f32)
            nc.vector.memset(partials, 0.0)

            # operand DMAs for BOTH batches up front; the two HWDGE
            # queues split the work.
            e_x = nc.scalar
            e_y = nc.sync
            lhsTs, rhss = [], []
            for b in range(BPC):
                lhsT = opnds.tile([13, P1], bf16)
                rhs = opnds.tile([13, P2], bf16)
                if b == 0:
                    # split batch 0's DMAs so the first matmuls (needing
                    # lhsT[:, :128] and rhs[:, :512]) start ~1us sooner
                    e_y.dma_start(out=lhsT[:, 0:256], in_=xprod_h[b][:, 0:256])
                    e_x.dma_start(out=rhs[:, 0:1024], in_=yprod_h[b][:, 0:1024])
                    e_y.dma_start(out=lhsT[:, 256:P1], in_=xprod_h[b][:, 256:P1])
                    e_x.dma_start(out=rhs[:, 1024:P2], in_=yprod_h[b][:, 1024:P2])
                else:
                    e_y.dma_start(out=lhsT[:], in_=xprod_h[b])
                    e_x.dma_start(out=rhs[:], in_=yprod_h[b])
                lhsTs.append(lhsT)
                rhss.append(rhs)
            mks = []
            for b in range(BPC):
                mk = small.tile([128, NC128], f32, tag="mk", bufs=2)
                nc.sync.dma_start(out=mk[:], in_=maskT_h[b])
                mks.append(mk)

            # mean(pred_dw^2) partial: behind the operand DMAs in the
            # queue (those gate the first matmuls); the square runs during
            # the startup window and pre-loads the ACT table.
            dwt = consts.tile([128, BPC * 48], f32)
            nc.scalar.dma_start(out=dwt[:], in_=dw_h[:])
            dwsq = consts.tile([128, BPC * 48], f32)
            nc.scalar.activation(
                out=dwsq[:], in_=dwt[:], func=Act.Square,
                accum_out=partials[:, 6:7],
            )

            # ---------- main distance + min pipeline ----------
            # No PE warm-up burst: while the HAM clock-gate is cold the PE
            # still outruns the ACT evacuation (1.7us vs 2.0us per group),
            # so cold mains cost nothing and start ~4us earlier.
            colaccs, rowparts_l = [], []

            def emit_finals(b):
                """Fold batch b's accumulators to per-batch scalars."""
                colacc = colaccs[b]
                rowparts = rowparts_l[b]
                rowaccs = small.tile([128, NT], f32)
                chamv = small.tile([128, NC128], f32)
                # finish row-min: [128, 16, 128] -> [128, 16] via bf16 TT
                # folds (2x mode) + a short 1x reduce — ~35% faster than a
                # single 1x tensor_reduce over FD=2048.
                ra1 = small.tile([128, NT, 64], bf16, tag="ra1", bufs=1)
                nc.vector.tensor_tensor(
                    out=ra1[:], in0=rowparts[:, :, 0:64],
                    in1=rowparts[:, :, 64:128], op=Alu.min,
                )
                ra2 = small.tile([128, NT, 16], bf16, tag="ra2", bufs=1)
                nc.vector.tensor_tensor(
                    out=ra2[:], in0=ra1[:, :, 0:16], in1=ra1[:, :, 16:32],
                    op=Alu.min,
                )
                nc.vector.tensor_tensor(
                    out=ra2[:], in0=ra2[:], in1=ra1[:, :, 32:48], op=Alu.min,
                )
                nc.vector.tensor_tensor(
                    out=ra2[:], in0=ra2[:], in1=ra1[:, :, 48:64], op=Alu.min,
                )
                nc.vector.tensor_reduce(
                    out=rowaccs[:], in_=ra2[:], axis=X, op=Alu.min
                )
                # fold colacc partitions via PE transpose
                for r in range(2):
                    tp = ps.tile([128, P2], bf16, tag="dgrp")
                    for cc in range(8):
                        cidx = r * 8 + cc
                        nc.tensor.transpose(
                            tp[:, cc * 128:(cc + 1) * 128],
                            colacc[:, cidx * 128:(cidx + 1) * 128],
                            ident[:],
                        )
                    tpv = tp[:, 0:1024].rearrange("p (a b) -> p a b", b=128)
                    nc.vector.tensor_reduce(
                        out=chamv[:, r * 8:(r + 1) * 8], in_=tpv, axis=X,
                        op=Alu.min,
                    )
                # per-batch scalars
                prod = small.tile([128, NC128], f32)
                nc.vector.tensor_tensor(
                    out=prod[:], in0=chamv[:], in1=mks[b][:], op=Alu.mult
                )
                nc.vector.tensor_reduce(
                    out=partials[:, 2 * b:2 * b + 1], in_=prod[:], axis=X,
                    op=Alu.add,
                )
                nc.vector.tensor_reduce(
                    out=partials[:, 2 * b + 1:2 * b + 2], in_=rowaccs[:],
                    axis=X, op=Alu.add,
                )
            for b in range(BPC):
                lhsT, rhs = lhsTs[b], rhss[b]
                colacc = opnds.tile([128, P2], bf16)
                rowparts = opnds.tile([128, NT, 128], bf16)
                colaccs.append(colacc)
                rowparts_l.append(rowparts)

                # ramp in with single/pair steps (shortest pipeline fill;
                # a quad's fold must wait for 4 evacuations), then
                # quad-steps (less DVE per-op overhead) once flowing.
                if b == 0:
                    steps = [[0], [1], [2, 3]] + [
                        list(range(s, s + 4)) for s in range(4, NT, 4)
                    ]
                else:
                    steps = [list(range(s, s + 4)) for s in range(0, NT, 4)]
                first = True
                for si, tiles in enumerate(steps):
                    last_step = b == BPC - 1 and si == len(steps) - 1
                    nu = len(tiles)
                    nb = 3 if nu == 4 else 2
                    s4 = scr.tile([128, nu, P2], bf16, tag=f"s{nu}", bufs=nb)
                    for u, t in enumerate(tiles):
                        g = ps.tile([128, P2], f32, tag="dgrp")
                        lsl = lhsT[:, t * 128:(t + 1) * 128]
                        for c in range(NJ):
                            sl = slice(c * 512, (c + 1) * 512)
                            nc.tensor.matmul(g[:, sl], lsl, rhs[:, sl])
                        # ACT evacuates PSUM -> SBUF bf16.  For the very
                        # first groups, evacuate per 512-chunk so each copy
                        # starts right after its matmul instead of waiting
                        # for the whole group (cuts the DVE fill by ~2us).
                        if b == 0 and si < 2:
                            for c in range(NJ):
                                sl = slice(c * 512, (c + 1) * 512)
                                nc.scalar.copy(out=s4[:, u, sl], in_=g[:, sl])
                        else:
                            nc.scalar.copy(out=s4[:, u, :], in_=g[:])

                    def row_chain():
                        # merged row-min fold chain over the step's groups.
                        # For quads the first level is emitted per-PAIR so
                        # each DVE op waits on 2 evacuations, not 4
                        # (+58cyc, kills the step-transition bubbles).
                        f1 = scr.tile(
                            [128, nu, 1024], bf16, tag=f"f1{nu}", bufs=nb
                        )
                        if nu == 4:
                            nc.vector.tensor_tensor(
                                out=f1[:, 0:2, :], in0=s4[:, 0:2, 0:1024],
                                in1=s4[:, 0:2, 1024:2048], op=Alu.min,
                            )
                            nc.vector.tensor_tensor(
                                out=f1[:, 2:4, :], in0=s4[:, 2:4, 0:1024],
                                in1=s4[:, 2:4, 1024:2048], op=Alu.min,
                            )
                        else:
                            nc.vector.tensor_tensor(
                                out=f1[:], in0=s4[:, :, 0:1024],
                                in1=s4[:, :, 1024:2048], op=Alu.min,
                            )
                        f2 = scr.tile(
                            [128, nu, 512], bf16, tag=f"f2{nu}", bufs=nb
                        )
                        nc.vector.tensor_tensor(
                            out=f2[:], in0=f1[:, :, 0:512],
                            in1=f1[:, :, 512:1024], op=Alu.min,
                        )
                        f3 = scr.tile(
                            [128, nu, 256], bf16, tag=f"f3{nu}", bufs=nb
                        )
                        nc.vector.tensor_tensor(
                            out=f3[:], in0=f2[:, :, 0:256],
                            in1=f2[:, :, 256:512], op=Alu.min,
                        )
                        nc.vector.tensor_tensor(
                            out=rowparts[:, tiles[0]:tiles[0] + nu, :],
                            in0=f3[:, :, 0:128], in1=f3[:, :, 128:256],
                            op=Alu.min,
                        )

                    def col_chain():
                        nonlocal first
                        if nu == 4:
                            # per-pair first level (same reason as f1)
                            m2 = scr.tile([128, 2, P2], bf16, tag="m2")
                            nc.vector.tensor_tensor(
                                out=m2[:, 0:1, :], in0=s4[:, 0:1, :],
                                in1=s4[:, 1:2, :], op=Alu.min,
                            )
                            nc.vector.tensor_tensor(
                                out=m2[:, 1:2, :], in0=s4[:, 2:3, :],
                                in1=s4[:, 3:4, :], op=Alu.min,
                            )
                            mm = scr.tile([128, P2], bf16, tag="mm")
                            nc.vector.tensor_tensor(
                                out=mm[:], in0=m2[:, 0, :], in1=m2[:, 1, :],
                                op=Alu.min,
                            )
                        elif nu == 2:
                            mm = scr.tile([128, P2], bf16, tag="mm")
                            nc.vector.tensor_tensor(
                                out=mm[:], in0=s4[:, 0, :], in1=s4[:, 1, :],
                                op=Alu.min,
                            )
                        else:  # nu == 1
                            mm = None
                        src = s4[:, 0, :] if mm is None else mm[:]
                        if first:
                            nc.vector.tensor_copy(out=colacc[:], in_=src)
                            first = False
                        else:
                            nc.vector.tensor_tensor(
                                out=colacc[:], in0=colacc[:], in1=src,
                                op=Alu.min,
                            )

                    # on the very last step, finish the col path first so
                    # the final colacc transposes (PE) start ~2us earlier;
                    # the row chain then runs under them.
                    if last_step:
                        col_chain()
                        row_chain()
                    else:
                        row_chain()
                        col_chain()

            # finals after all mains: the PE processes its queue far ahead
            # of the DVE, so transposes emitted any earlier stall the
            # in-order PE queue waiting on colacc (measured: -7us).
            for b in range(BPC):
                emit_finals(b)

            # ---------- cross-partition sum of all partials via PE ----------
            fin = ps.tile([128, P2], f32, tag="dgrp")
            nc.tensor.matmul(fin[0:1, 0:8], ones128[:], partials[:])
            res = small.tile([1, 8], f32)
            nc.scalar.copy(res[:], fin[0:1, 0:8])
            nc.sync.dma_start(out=out_h[:], in_=res[:])

    nc.compile()
    return nc


def get_compiled():
    if "nc" not in _CACHE:
        _CACHE["nc"] = build_bass()
    return _CACHE["nc"]


def _hilo(a, bf16):
    """Lossless-ish bf16 hi/lo split of a float32 array."""
    hi = a.astype(bf16)
    lo = (a - hi.astype(np.float32)).astype(bf16)
    return hi, lo


def make_in_maps(v, v_pred, mask, pred_dw):
    import ml_dtypes

    bf16 = ml_dtypes.bfloat16
    v = np.asarray(v, np.float32)
    v_pred = np.asarray(v_pred, np.float32)
    mask = np.asarray(mask, np.float32)
    pred_dw = np.asarray(pred_dw, np.float32)

    # bf16 hi/lo repacking of the matmul operands, norms included
    wT = (-2.0 * v_pred).transpose(0, 2, 1)           # (16, 3, 2048) f32
    wh, wl = _hilo(wT, bf16)
    yT = v.transpose(0, 2, 1)
    yh, yl = _hilo(yT, bf16)
    x2 = (v_pred * v_pred).sum(-1)[:, None, :]        # (16, 1, 2048)
    x2h, x2l = _hilo(x2, bf16)
    y2 = (v * v).sum(-1)[:, None, :]
    y2h, y2l = _hilo(y2, bf16)
    ones = np.ones((B, 2, P1), dtype=bf16)
    # lhsT rows: [wh x3, wl x3, wh x3, 1, 1, x2h, x2l]
    xprod = np.concatenate([wh, wl, wh, ones, x2h, x2l], axis=1)  # (16,13,2048)
    # rhs rows:  [yh x3, yh x3, yl x3, y2h, y2l, 1, 1]
    yprod = np.concatenate([yh, yh, yl, y2h, y2l, ones], axis=1)

    mask_flat = mask.reshape(B, P2)
    # maskT[b, p, c] = mask_flat[b, c*128 + p]
    maskT = np.ascontiguousarray(
        mask_flat.reshape(B, NC128, 128).transpose(0, 2, 1)
    )
    in_maps = []
    for k in range(NCORES):
        b0 = BPC * k
        dwp = np.concatenate(
            [pred_dw[b0 + i].reshape(128, 48) for i in range(BPC)], axis=1
        )
        in_maps.append({
            "xprod": np.ascontiguousarray(xprod[b0:b0 + BPC]),
            "yprod": np.ascontiguousarray(yprod[b0:b0 + BPC]),
            "maskT": np.ascontiguousarray(maskT[b0:b0 + BPC]),
            "dw": np.ascontiguousarray(dwp),
        })
    return in_maps


def combine_outs(outs):
    """outs: (8, 8) array of per-core partial rows -> (loss, loss_normals)."""
    outs = np.asarray(outs, np.float64)
    mcols = [2 * i for i in range(BPC)]
    rcols = [2 * i + 1 for i in range(BPC)]
    msum = outs[:, mcols].sum()
    rsum = outs[:, rcols].sum()
    dsum = outs[:, 6].sum()
    loss = msum / (B * P2) + rsum / (B * P1) + dsum / (B * P1 * D)
    return (np.float32(loss), np.float32(0.0))


def kernel(**inputs):
    from concourse.bass_utils import run_bass_kernel_spmd

    nc = get_compiled()
    in_maps = make_in_maps(
        inputs["v"], inputs["v_pred"], inputs["mask"], inputs["pred_dw"]
    )
    res = run_bass_kernel_spmd(nc, in_maps, core_ids=list(range(NCORES)))
    outs = np.stack([r["out"].reshape(8) for r in res.results])
    return combine_outs(outs)


# revision 24
# speedup vs baseline: 1.0418x; 1.0418x over previous
"""Chamfer-distance loss (CCHLoss) kernel for 8 Trainium2 NeuronCores.

Contract: kernel(**inputs) takes the FULL unsharded inputs
  v:        (16, 2048, 3) f32
  v_pred:   (16, 2048, 3) f32
  mask:     (4, 4, 2, 32, 32) f32
  pred_dw:  (16, 2048, 3) f32
and returns (loss, loss_normals) matching reference().

Strategy: data-parallel over the B=16 batch dim, 2 batches per core.
Per batch the 2048x2048 squared-distance matrix is produced by TensorE
via a K=13 matmul (bf16 hi/lo split of both the -2x*y products and the
|x|^2 / |y|^2 norm rows, all packed on HOST) in 16 PSUM groups of
[128, 2048].  ScalarE evacuates each group to SBUF bf16; VectorE
reduces it with tensor_tensor min trees:
  - per group a fold chain 2048->128 plus one tensor_reduce gives the
    row-min (-> cham_pred),
  - a bf16 min pyramid accumulates the column-min (-> colacc),
  - PE transposes + reduce fold colacc's 128 partitions (-> cham_v),
  - mask-weighted sums reduce everything to per-core scalars.
Host only shards/packs inputs and sums 8 cores' partial sums.
"""

import numpy as np

B, P1, P2, D = 16, 2048, 2048, 3
NCORES = 8
BPC = B // NCORES  # batches per core
NT = P1 // 128     # i-tiles per batch
NJ = P2 // 512     # matmul j-chunks per group
NC128 = P2 // 128  # 128-wide j-chunks (transpose fold)

_CACHE = {}


def build_bass():
    """Build + compile the per-core Bass program (same program all 8 cores)."""
    import concourse.bacc as bacc
    import concourse.tile as tile
    from concourse import mybir
    from concourse.masks import make_identity

    f32 = mybir.dt.float32
    bf16 = mybir.dt.bfloat16
    Alu = mybir.AluOpType
    Act = mybir.ActivationFunctionType
    X = mybir.AxisListType.X

    nc = bacc.Bacc("TRN2", target_bir_lowering=False, debug=False)

    xprod_h = nc.dram_tensor("xprod", (BPC, 13, P1), bf16, kind="ExternalInput")
    yprod_h = nc.dram_tensor("yprod", (BPC, 13, P2), bf16, kind="ExternalInput")
    maskT_h = nc.dram_tensor("maskT", (BPC, 128, NC128), f32, kind="ExternalInput")
    dw_h = nc.dram_tensor("dw", (128, BPC * 48), f32, kind="ExternalInput")
    out_h = nc.dram_tensor("out", (1, 8), f32, kind="ExternalOutput")

    with tile.TileContext(nc) as tc:
        with (
            tc.tile_pool(name="consts", bufs=1) as consts,
            tc.tile_pool(name="opnds", bufs=2) as opnds,
            tc.tile_pool(name="scr", bufs=3) as scr,
            tc.tile_pool(name="small", bufs=4) as small,
            tc.tile_pool(name="ps", bufs=2, space="PSUM") as ps,
        ):
            ident = consts.tile([128, 128], bf16)
            make_identity(nc, ident)
            ones128 = consts.tile([128, 1], f32)
            nc.vector.memset(ones128, 1.0)
            partials = consts.tile([128, 8], f32)
            nc.vector.memset(partials, 0.0)

            # operand DMAs for BOTH batches up front; the two HWDGE
            # queues split the work.
            e_x = nc.scalar
            e_y = nc.sync
            lhsTs, rhss = [], []
            for b in range(BPC):
                lhsT = opnds.tile([13, P1], bf16)
                rhs = opnds.tile([13, P2], bf16)
                if b == 0:
                    # split batch 0's DMAs so the first matmuls (needing
                    # lhsT[:, :128] and rhs[:, :512]) start ~1us sooner
                    e_y.dma_start(out=lhsT[:, 0:256], in_=xprod_h[b][:, 0:256])
                    e_x.dma_start(out=rhs[:, 0:1024], in_=yprod_h[b][:, 0:1024])
                    e_y.dma_start(out=lhsT[:, 256:P1], in_=xprod_h[b][:, 256:P1])
                    e_x.dma_start(out=rhs[:, 1024:P2], in_=yprod_h[b][:, 1024:P2])
                else:
                    e_y.dma_start(out=lhsT[:], in_=xprod_h[b])
                    e_x.dma_start(out=rhs[:], in_=yprod_h[b])
                lhsTs.append(lhsT)
                rhss.append(rhs)
            mks = []
            for b in range(BPC):
                mk = small.tile([128, NC128], f32, tag="mk", bufs=2)
                nc.sync.dma_start(out=mk[:], in_=maskT_h[b])
                mks.append(mk)

            # tiny warm activation with no data deps: pulls the ~1.3us
            # ACT_TABLE_LOAD to t=0 so the first evacuation isn't delayed
            warm = consts.tile([1, 1], f32)
            nc.vector.memset(warm, 0.0)
            warmo = consts.tile([1, 1], f32)
            nc.scalar.activation(out=warmo[:], in_=warm[:], func=Act.Square)

            # ---------- main distance + min pipeline ----------
            # No PE warm-up burst: while the HAM clock-gate is cold the PE
            # still outruns the ACT evacuation (1.7us vs 2.0us per group),
            # so cold mains cost nothing and start ~4us earlier.
            colaccs, rowparts_l = [], []

            def emit_finals(b):
                """Fold batch b's accumulators to per-batch scalars."""
                colacc = colaccs[b]
                rowparts = rowparts_l[b]
                rowaccs = small.tile([128, NT], f32)
                chamv = small.tile([128, NC128], f32)
                # finish row-min: [128, 16, 128] -> [128, 16] via bf16 TT
                # folds (2x mode) + a short 1x reduce — ~35% faster than a
                # single 1x tensor_reduce over FD=2048.
                ra1 = small.tile([128, NT, 64], bf16, tag="ra1", bufs=1)
                nc.vector.tensor_tensor(
                    out=ra1[:], in0=rowparts[:, :, 0:64],
                    in1=rowparts[:, :, 64:128], op=Alu.min,
                )
                ra2 = small.tile([128, NT, 16], bf16, tag="ra2", bufs=1)
                nc.vector.tensor_tensor(
                    out=ra2[:], in0=ra1[:, :, 0:16], in1=ra1[:, :, 16:32],
                    op=Alu.min,
                )
                nc.vector.tensor_tensor(
                    out=ra2[:], in0=ra2[:], in1=ra1[:, :, 32:48], op=Alu.min,
                )
                nc.vector.tensor_tensor(
                    out=ra2[:], in0=ra2[:], in1=ra1[:, :, 48:64], op=Alu.min,
                )
                nc.vector.tensor_reduce(
                    out=rowaccs[:], in_=ra2[:], axis=X, op=Alu.min
                )
                # fold colacc partitions via PE transpose
                for r in range(2):
                    tp = ps.tile([128, P2], bf16, tag="dgrp")
                    for cc in range(8):
                        cidx = r * 8 + cc
                        nc.tensor.transpose(
                            tp[:, cc * 128:(cc + 1) * 128],
                            colacc[:, cidx * 128:(cidx + 1) * 128],
                            ident[:],
                        )
                    tpv = tp[:, 0:1024].rearrange("p (a b) -> p a b", b=128)
                    nc.vector.tensor_reduce(
                        out=chamv[:, r * 8:(r + 1) * 8], in_=tpv, axis=X,
                        op=Alu.min,
                    )
                # per-batch scalars
                prod = small.tile([128, NC128], f32)
                nc.vector.tensor_tensor(
                    out=prod[:], in0=chamv[:], in1=mks[b][:], op=Alu.mult
                )
                nc.vector.tensor_reduce(
                    out=partials[:, 2 * b:2 * b + 1], in_=prod[:], axis=X,
                    op=Alu.add,
                )
                nc.vector.tensor_reduce(
                    out=partials[:, 2 * b + 1:2 * b + 2], in_=rowaccs[:],
                    axis=X, op=Alu.add,
                )
            for b in range(BPC):
                lhsT, rhs = lhsTs[b], rhss[b]
                colacc = opnds.tile([128, P2], bf16)
                rowparts = opnds.tile([128, NT, 128], bf16)
                colaccs.append(colacc)
                rowparts_l.append(rowparts)

                # ramp in with single/pair steps (shortest pipeline fill;
                # a quad's fold must wait for 4 evacuations), then
                # quad-steps (less DVE per-op overhead) once flowing.
                if b == 0:
                    steps = [[0], [1], [2, 3]] + [
                        list(range(s, s + 4)) for s in range(4, NT, 4)
                    ]
                else:
                    steps = [list(range(s, s + 4)) for s in range(0, NT, 4)]
                first = True
                for si, tiles in enumerate(steps):
                    last_step = b == BPC - 1 and si == len(steps) - 1
                    nu = len(tiles)
                    nb = 3 if nu == 4 else 2
                    s4 = scr.tile([128, nu, P2], bf16, tag=f"s{nu}", bufs=nb)
                    for u, t in enumerate(tiles):
                        g = ps.tile([128, P2], f32, tag="dgrp")
                        lsl = lhsT[:, t * 128:(t + 1) * 128]
                        for c in range(NJ):
                            sl = slice(c * 512, (c + 1) * 512)
                            nc.tensor.matmul(g[:, sl], lsl, rhs[:, sl])
                        # ACT evacuates PSUM -> SBUF bf16 (monolithic:
                        # PSUM dep-tracking is per-tile, so chunked copies
                        # still wait on all 4 matmuls and only add time)
                        nc.scalar.copy(out=s4[:, u, :], in_=g[:])

                    def row_chain():
                        # merged row-min fold chain over the step's groups.
                        # For quads the first level is emitted per-PAIR so
                        # each DVE op waits on 2 evacuations, not 4
                        # (+58cyc, kills the step-transition bubbles).
                        f1 = scr.tile(
                            [128, nu, 1024], bf16, tag=f"f1{nu}", bufs=nb
                        )
                        if nu == 4:
                            nc.vector.tensor_tensor(
                                out=f1[:, 0:2, :], in0=s4[:, 0:2, 0:1024],
                                in1=s4[:, 0:2, 1024:2048], op=Alu.min,
                            )
                            nc.vector.tensor_tensor(
                                out=f1[:, 2:4, :], in0=s4[:, 2:4, 0:1024],
                                in1=s4[:, 2:4, 1024:2048], op=Alu.min,
                            )
                        else:
                            nc.vector.tensor_tensor(
                                out=f1[:], in0=s4[:, :, 0:1024],
                                in1=s4[:, :, 1024:2048], op=Alu.min,
                            )
                        f2 = scr.tile(
                            [128, nu, 512], bf16, tag=f"f2{nu}", bufs=nb
                        )
                        nc.vector.tensor_tensor(
                            out=f2[:], in0=f1[:, :, 0:512],
                            in1=f1[:, :, 512:1024], op=Alu.min,
                        )
                        f3 = scr.tile(
                            [128, nu, 256], bf16, tag=f"f3{nu}", bufs=nb
                        )
                        nc.vector.tensor_tensor(
                            out=f3[:], in0=f2[:, :, 0:256],
                            in1=f2[:, :, 256:512], op=Alu.min,
                        )
                        nc.vector.tensor_tensor(
                            out=rowparts[:, tiles[0]:tiles[0] + nu, :],
                            in0=f3[:, :, 0:128], in1=f3[:, :, 128:256],
                            op=Alu.min,
                        )

                    def col_chain():
                        nonlocal first
                        if nu == 4:
                            # per-pair first level (same reason as f1)
                            m2 = scr.tile([128, 2, P2], bf16, tag="m2")
                            nc.vector.tensor_tensor(
                                out=m2[:, 0:1, :], in0=s4[:, 0:1, :],
                                in1=s4[:, 1:2, :], op=Alu.min,
                            )
                            nc.vector.tensor_tensor(
                                out=m2[:, 1:2, :], in0=s4[:, 2:3, :],
                                in1=s4[:, 3:4, :], op=Alu.min,
                            )
                            if first:
                                # fold straight into colacc: saves the
                                # separate init copy
                                nc.vector.tensor_tensor(
                                    out=colacc[:], in0=m2[:, 0, :],
                                    in1=m2[:, 1, :], op=Alu.min,
                                )
                                first = False
                                return
                            mm = scr.tile([128, P2], bf16, tag="mm")
                            nc.vector.tensor_tensor(
                                out=mm[:], in0=m2[:, 0, :], in1=m2[:, 1, :],
                                op=Alu.min,
                            )
                        elif nu == 2:
                            if first:
                                nc.vector.tensor_tensor(
                                    out=colacc[:], in0=s4[:, 0, :],
                                    in1=s4[:, 1, :], op=Alu.min,
                                )
                                first = False
                                return
                            mm = scr.tile([128, P2], bf16, tag="mm")
                            nc.vector.tensor_tensor(
                                out=mm[:], in0=s4[:, 0, :], in1=s4[:, 1, :],
                                op=Alu.min,
                            )
                        else:  # nu == 1
                            mm = None
                        src = s4[:, 0, :] if mm is None else mm[:]
                        if first:
                            nc.vector.tensor_copy(out=colacc[:], in_=src)
                            first = False
                        else:
                            nc.vector.tensor_tensor(
                                out=colacc[:], in0=colacc[:], in1=src,
                                op=Alu.min,
                            )

                    # on the very last step, finish the col path first so
                    # the final colacc transposes (PE) start ~2us earlier;
                    # the row chain then runs under them.
                    if last_step:
                        col_chain()
                        row_chain()
                    else:
                        row_chain()
                        col_chain()

            # finals after all mains: the PE processes its queue far ahead
            # of the DVE, so transposes emitted any earlier stall the
            # in-order PE queue waiting on colacc (measured: -7us).
            for b in range(BPC):
                emit_finals(b)

            # mean(pred_dw^2) partial: ACT is idle for the last ~20us of
            # the span, so this costs nothing here; emitting it earlier
            # delays the evacuation stream.
            dwt = consts.tile([128, BPC * 48], f32)
            nc.sync.dma_start(out=dwt[:], in_=dw_h[:])
            dwsq = consts.tile([128, BPC * 48], f32)
            nc.scalar.activation(
                out=dwsq[:], in_=dwt[:], func=Act.Square,
                accum_out=partials[:, 6:7],
            )

            # ---------- cross-partition sum of all partials via PE ----------
            fin = ps.tile([128, P2], f32, tag="dgrp")
            nc.tensor.matmul(fin[0:1, 0:8], ones128[:], partials[:])
            res = small.tile([1, 8], f32)
            nc.scalar.copy(res[:], fin[0:1, 0:8])
            nc.sync.dma_start(out=out_h[:], in_=res[:])

    nc.compile()
    return nc


def get_compiled():
    if "nc" not in _CACHE:
        _CACHE["nc"] = build_bass()
    return _CACHE["nc"]


def _hilo(a, bf16):
    """Lossless-ish bf16 hi/lo split of a float32 array."""
    hi = a.astype(bf16)
    lo = (a - hi.astype(np.float32)).astype(bf16)
    return hi, lo


def make_in_maps(v, v_pred, mask, pred_dw):
    import ml_dtypes

    bf16 = ml_dtypes.bfloat16
    v = np.asarray(v, np.float32)
    v_pred = np.asarray(v_pred, np.float32)
    mask = np.asarray(mask, np.float32)
    pred_dw = np.asarray(pred_dw, np.float32)

    # bf16 hi/lo repacking of the matmul operands, norms included
    wT = (-2.0 * v_pred).transpose(0, 2, 1)           # (16, 3, 2048) f32
    wh, wl = _hilo(wT, bf16)
    yT = v.transpose(0, 2, 1)
    yh, yl = _hilo(yT, bf16)
    x2 = (v_pred * v_pred).sum(-1)[:, None, :]        # (16, 1, 2048)
    x2h, x2l = _hilo(x2, bf16)
    y2 = (v * v).sum(-1)[:, None, :]
    y2h, y2l = _hilo(y2, bf16)
    ones = np.ones((B, 2, P1), dtype=bf16)
    # lhsT rows: [wh x3, wl x3, wh x3, 1, 1, x2h, x2l]
    xprod = np.concatenate([wh, wl, wh, ones, x2h, x2l], axis=1)  # (16,13,2048)
    # rhs rows:  [yh x3, yh x3, yl x3, y2h, y2l, 1, 1]
    yprod = np.concatenate([yh, yh, yl, y2h, y2l, ones], axis=1)

    mask_flat = mask.reshape(B, P2)
    # maskT[b, p, c] = mask_flat[b, c*128 + p]
    maskT = np.ascontiguousarray(
        mask_flat.reshape(B, NC128, 128).transpose(0, 2, 1)
    )
    in_maps = []
    for k in range(NCORES):
        b0 = BPC * k
        dwp = np.concatenate(
            [pred_dw[b0 + i].reshape(128, 48) for i in range(BPC)], axis=1
        )
        in_maps.append({
            "xprod": np.ascontiguousarray(xprod[b0:b0 + BPC]),
            "yprod": np.ascontiguousarray(yprod[b0:b0 + BPC]),
            "maskT": np.ascontiguousarray(maskT[b0:b0 + BPC]),
            "dw": np.ascontiguousarray(dwp),
        })
    return in_maps


def combine_outs(outs):
    """outs: (8, 8) array of per-core partial rows -> (loss, loss_normals)."""
    outs = np.asarray(outs, np.float64)
    mcols = [2 * i for i in range(BPC)]
    rcols = [2 * i + 1 for i in range(BPC)]
    msum = outs[:, mcols].sum()
    rsum = outs[:, rcols].sum()
    dsum = outs[:, 6].sum()
    loss = msum / (B * P2) + rsum / (B * P1) + dsum / (B * P1 * D)
    return (np.float32(loss), np.float32(0.0))


def kernel(**inputs):
    from concourse.bass_utils import run_bass_kernel_spmd

    nc = get_compiled()
    in_maps = make_in_maps(
        inputs["v"], inputs["v_pred"], inputs["mask"], inputs["pred_dw"]
    )
    res = run_bass_kernel_spmd(nc, in_maps, core_ids=list(range(NCORES)))
    outs = np.stack([r["out"].reshape(8) for r in res.results])
    return combine_outs(outs)


# revision 38
# speedup vs baseline: 1.0477x; 1.0056x over previous
"""Chamfer-distance loss (CCHLoss) kernel for 8 Trainium2 NeuronCores.

Contract: kernel(**inputs) takes the FULL unsharded inputs
  v:        (16, 2048, 3) f32
  v_pred:   (16, 2048, 3) f32
  mask:     (4, 4, 2, 32, 32) f32
  pred_dw:  (16, 2048, 3) f32
and returns (loss, loss_normals) matching reference().

Strategy: data-parallel over the B=16 batch dim, 2 batches per core.
Per batch the 2048x2048 squared-distance matrix is produced by TensorE
via a K=13 matmul (bf16 hi/lo split of both the -2x*y products and the
|x|^2 / |y|^2 norm rows, all packed on HOST) in 16 PSUM groups of
[128, 2048].  ScalarE evacuates each group to SBUF bf16; VectorE
reduces it with tensor_tensor min trees:
  - per group a fold chain 2048->128 plus one tensor_reduce gives the
    row-min (-> cham_pred),
  - a bf16 min pyramid accumulates the column-min (-> colacc),
  - PE transposes + reduce fold colacc's 128 partitions (-> cham_v),
  - mask-weighted sums reduce everything to per-core scalars.
Host only shards/packs inputs and sums 8 cores' partial sums.
"""

import numpy as np

B, P1, P2, D = 16, 2048, 2048, 3
NCORES = 8
BPC = B // NCORES  # batches per core
NT = P1 // 128     # i-tiles per batch
NJ = P2 // 512     # matmul j-chunks per group
NC128 = P2 // 128  # 128-wide j-chunks (transpose fold)

_CACHE = {}


def build_bass():
    """Build + compile the per-core Bass program (same program all 8 cores)."""
    import concourse.bacc as bacc
    import concourse.tile as tile
    from concourse import mybir
    from concourse.masks import make_identity

    f32 = mybir.dt.float32
    bf16 = mybir.dt.bfloat16
    Alu = mybir.AluOpType
    Act = mybir.ActivationFunctionType
    X = mybir.AxisListType.X

    nc = bacc.Bacc("TRN2", target_bir_lowering=False, debug=False)

    xprod_h = nc.dram_tensor("xprod", (BPC, 13, P1), bf16, kind="ExternalInput")
    yprod_h = nc.dram_tensor("yprod", (BPC, 13, P2), bf16, kind="ExternalInput")
    maskT_h = nc.dram_tensor("maskT", (BPC, 128, NC128), f32, kind="ExternalInput")
    dw_h = nc.dram_tensor("dw", (128, BPC * 48), f32, kind="ExternalInput")
    # per-partition partials go to the host unsummed: dropping the final
    # PE ones-matmul + res copy shortens the tail by ~0.6us
    out_h = nc.dram_tensor("out", (128, 8), f32, kind="ExternalOutput")

    with tile.TileContext(nc) as tc:
        with (
            tc.tile_pool(name="consts", bufs=1) as consts,
            tc.tile_pool(name="opnds", bufs=2) as opnds,
            tc.tile_pool(name="scr", bufs=3) as scr,
            tc.tile_pool(name="small", bufs=4) as small,
            tc.tile_pool(name="ps", bufs=2, space="PSUM") as ps,
        ):
            ident = consts.tile([128, 128], bf16)
            make_identity(nc, ident)
            partials = consts.tile([128, 8], f32)
            nc.vector.memset(partials, 0.0)

            # operand DMAs for BOTH batches up front; the two HWDGE
            # queues split the work.
            e_x = nc.scalar
            e_y = nc.sync
            lhsTs, rhss = [], []
            for b in range(BPC):
                lhsT = opnds.tile([13, P1], bf16)
                rhs = opnds.tile([13, P2], bf16)
                if b == 0:
                    # split batch 0's DMAs so the first matmuls (needing
                    # lhsT[:, :128] and rhs[:, :512]) start ~1us sooner
                    e_y.dma_start(out=lhsT[:, 0:256], in_=xprod_h[b][:, 0:256])
                    e_x.dma_start(out=rhs[:, 0:1024], in_=yprod_h[b][:, 0:1024])
                    e_y.dma_start(out=lhsT[:, 256:P1], in_=xprod_h[b][:, 256:P1])
                    e_x.dma_start(out=rhs[:, 1024:P2], in_=yprod_h[b][:, 1024:P2])
                else:
                    e_y.dma_start(out=lhsT[:], in_=xprod_h[b])
                    e_x.dma_start(out=rhs[:], in_=yprod_h[b])
                lhsTs.append(lhsT)
                rhss.append(rhs)
            mks = []
            for b in range(BPC):
                mk = small.tile([128, NC128], f32, tag="mk", bufs=2)
                nc.sync.dma_start(out=mk[:], in_=maskT_h[b])
                mks.append(mk)

            # tiny warm activation with no data deps: pulls the ~1.3us
            # ACT_TABLE_LOAD to t=0 so the first evacuation isn't delayed
            warm = consts.tile([1, 1], f32)
            nc.vector.memset(warm, 0.0)
            warmo = consts.tile([1, 1], f32)
            nc.scalar.activation(out=warmo[:], in_=warm[:], func=Act.Square)

            # small PE warm-up burst sized to fit INSIDE the ~2.7us input
            # DMA wait: the HAM clock-gate reaches 2.4 GHz sooner without
            # delaying the first real matmul.
            wsrc = consts.tile([13, 256], bf16)
            nc.vector.memset(wsrc, 1.0)
            wps = ps.tile([128, P2], f32, tag="dgrp")
            for _ in range(8):
                nc.tensor.matmul(wps[:, 0:256], wsrc[:, 0:128], wsrc[:])

            # ---------- main distance + min pipeline ----------
            # No PE warm-up burst: while the HAM clock-gate is cold the PE
            # still outruns the ACT evacuation (1.7us vs 2.0us per group),
            # so cold mains cost nothing and start ~4us earlier.
            colaccs, rowparts_l = [], []

            def emit_finals(b):
                """Fold batch b's accumulators to per-batch scalars."""
                colacc = colaccs[b]
                rowparts = rowparts_l[b]
                rowaccs = small.tile([128, NT], f32, bufs=2)
                chamv = small.tile([128, NC128], f32, bufs=2)
                # finish row-min: [128, 16, 128] -> [128, 16] via bf16 TT
                # folds (2x mode) + a short 1x reduce — ~35% faster than a
                # single 1x tensor_reduce over FD=2048.
                ra1 = small.tile([128, NT, 64], bf16, tag="ra1", bufs=1)
                nc.vector.tensor_tensor(
                    out=ra1[:], in0=rowparts[:, :, 0:64],
                    in1=rowparts[:, :, 64:128], op=Alu.min,
                )
                ra2 = small.tile([128, NT, 16], bf16, tag="ra2", bufs=1)
                nc.vector.tensor_tensor(
                    out=ra2[:], in0=ra1[:, :, 0:16], in1=ra1[:, :, 16:32],
                    op=Alu.min,
                )
                nc.vector.tensor_tensor(
                    out=ra2[:], in0=ra2[:], in1=ra1[:, :, 32:48], op=Alu.min,
                )
                nc.vector.tensor_tensor(
                    out=ra2[:], in0=ra2[:], in1=ra1[:, :, 48:64], op=Alu.min,
                )
                nc.vector.tensor_reduce(
                    out=rowaccs[:], in_=ra2[:], axis=X, op=Alu.min
                )
                # fold colacc partitions via PE transpose
                for r in range(2):
                    tp = ps.tile([128, P2], bf16, tag="dgrp")
                    for cc in range(8):
                        cidx = r * 8 + cc
                        nc.tensor.transpose(
                            tp[:, cc * 128:(cc + 1) * 128],
                            colacc[:, cidx * 128:(cidx + 1) * 128],
                            ident[:],
                        )
                    tpv = tp[:, 0:1024].rearrange("p (a b) -> p a b", b=128)
                    nc.vector.tensor_reduce(
                        out=chamv[:, r * 8:(r + 1) * 8], in_=tpv, axis=X,
                        op=Alu.min,
                    )
                # per-batch scalars
                prod = small.tile([128, NC128], f32, bufs=2)
                nc.vector.tensor_tensor(
                    out=prod[:], in0=chamv[:], in1=mks[b][:], op=Alu.mult
                )
                nc.vector.tensor_reduce(
                    out=partials[:, 2 * b:2 * b + 1], in_=prod[:], axis=X,
                    op=Alu.add,
                )
                nc.vector.tensor_reduce(
                    out=partials[:, 2 * b + 1:2 * b + 2], in_=rowaccs[:],
                    axis=X, op=Alu.add,
                )
            for b in range(BPC):
                lhsT, rhs = lhsTs[b], rhss[b]
                colacc = opnds.tile([128, P2], bf16)
                rowparts = opnds.tile([128, NT, 128], bf16)
                colaccs.append(colacc)
                rowparts_l.append(rowparts)

                # ramp in with single/pair steps (shortest pipeline fill;
                # a quad's fold must wait for 4 evacuations; pairs also let
                # the slower-per-group DVE fall behind ACT gradually so the
                # first quads never stall), then quad-steps once flowing.
                if b == 0:
                    steps = [[0], [1], [2, 3], [4, 5], [6, 7]] + [
                        list(range(s, s + 4)) for s in range(8, NT, 4)
                    ]
                else:
                    steps = [list(range(s, s + 4)) for s in range(0, NT, 4)]
                first = True
                for si, tiles in enumerate(steps):
                    last_step = b == BPC - 1 and si == len(steps) - 1
                    nu = len(tiles)
                    nb = 3 if nu == 4 else 2
                    s4 = scr.tile([128, nu, P2], bf16, tag=f"s{nu}", bufs=nb)
                    for u, t in enumerate(tiles):
                        g = ps.tile([128, P2], f32, tag="dgrp")
                        lsl = lhsT[:, t * 128:(t + 1) * 128]
                        for c in range(NJ):
                            sl = slice(c * 512, (c + 1) * 512)
                            nc.tensor.matmul(g[:, sl], lsl, rhs[:, sl])
                        # ACT evacuates PSUM -> SBUF bf16 (monolithic:
                        # PSUM dep-tracking is per-tile, so chunked copies
                        # still wait on all 4 matmuls and only add time)
                        nc.scalar.copy(out=s4[:, u, :], in_=g[:])

                    def row_chain():
                        # merged row-min fold chain over the step's groups.
                        # For quads the first level is emitted per-PAIR so
                        # each DVE op waits on 2 evacuations, not 4
                        # (+58cyc, kills the step-transition bubbles).
                        f1 = scr.tile(
                            [128, nu, 1024], bf16, tag=f"f1{nu}", bufs=nb
                        )
                        if nu == 4:
                            nc.vector.tensor_tensor(
                                out=f1[:, 0:2, :], in0=s4[:, 0:2, 0:1024],
                                in1=s4[:, 0:2, 1024:2048], op=Alu.min,
                            )
                            nc.vector.tensor_tensor(
                                out=f1[:, 2:4, :], in0=s4[:, 2:4, 0:1024],
                                in1=s4[:, 2:4, 1024:2048], op=Alu.min,
                            )
                        else:
                            nc.vector.tensor_tensor(
                                out=f1[:], in0=s4[:, :, 0:1024],
                                in1=s4[:, :, 1024:2048], op=Alu.min,
                            )
                        f2 = scr.tile(
                            [128, nu, 512], bf16, tag=f"f2{nu}", bufs=nb
                        )
                        nc.vector.tensor_tensor(
                            out=f2[:], in0=f1[:, :, 0:512],
                            in1=f1[:, :, 512:1024], op=Alu.min,
                        )
                        f3 = scr.tile(
                            [128, nu, 256], bf16, tag=f"f3{nu}", bufs=nb
                        )
                        nc.vector.tensor_tensor(
                            out=f3[:], in0=f2[:, :, 0:256],
                            in1=f2[:, :, 256:512], op=Alu.min,
                        )
                        nc.vector.tensor_tensor(
                            out=rowparts[:, tiles[0]:tiles[0] + nu, :],
                            in0=f3[:, :, 0:128], in1=f3[:, :, 128:256],
                            op=Alu.min,
                        )

                    def col_chain():
                        nonlocal first
                        if nu == 4:
                            # per-pair first level (same reason as f1)
                            m2 = scr.tile([128, 2, P2], bf16, tag="m2")
                            nc.vector.tensor_tensor(
                                out=m2[:, 0:1, :], in0=s4[:, 0:1, :],
                                in1=s4[:, 1:2, :], op=Alu.min,
                            )
                            nc.vector.tensor_tensor(
                                out=m2[:, 1:2, :], in0=s4[:, 2:3, :],
                                in1=s4[:, 3:4, :], op=Alu.min,
                            )
                            if first:
                                # fold straight into colacc: saves the
                                # separate init copy
                                nc.vector.tensor_tensor(
                                    out=colacc[:], in0=m2[:, 0, :],
                                    in1=m2[:, 1, :], op=Alu.min,
                                )
                                first = False
                                return
                            mm = scr.tile([128, P2], bf16, tag="mm")
                            nc.vector.tensor_tensor(
                                out=mm[:], in0=m2[:, 0, :], in1=m2[:, 1, :],
                                op=Alu.min,
                            )
                        elif nu == 2:
                            if first:
                                nc.vector.tensor_tensor(
                                    out=colacc[:], in0=s4[:, 0, :],
                                    in1=s4[:, 1, :], op=Alu.min,
                                )
                                first = False
                                return
                            mm = scr.tile([128, P2], bf16, tag="mm")
                            nc.vector.tensor_tensor(
                                out=mm[:], in0=s4[:, 0, :], in1=s4[:, 1, :],
                                op=Alu.min,
                            )
                        else:  # nu == 1
                            mm = None
                        src = s4[:, 0, :] if mm is None else mm[:]
                        if first:
                            nc.vector.tensor_copy(out=colacc[:], in_=src)
                            first = False
                        else:
                            nc.vector.tensor_tensor(
                                out=colacc[:], in0=colacc[:], in1=src,
                                op=Alu.min,
                            )

                    # on the very last step, finish the col path first so
                    # the final colacc transposes (PE) start ~2us earlier;
                    # the row chain then runs under them.
                    if last_step:
                        col_chain()
                        row_chain()
                    else:
                        row_chain()
                        col_chain()

            # finals after all mains: the PE processes its queue far ahead
            # of the DVE, so transposes emitted any earlier stall the
            # in-order PE queue waiting on colacc (measured: -7us).
            for b in range(BPC):
                emit_finals(b)

            # mean(pred_dw^2) partial: ACT is idle for the last ~20us of
            # the span, so this costs nothing here; emitting it earlier
            # delays the evacuation stream.
            dwt = consts.tile([128, BPC * 48], f32)
            nc.sync.dma_start(out=dwt[:], in_=dw_h[:])
            dwsq = consts.tile([128, BPC * 48], f32)
            nc.scalar.activation(
                out=dwsq[:], in_=dwt[:], func=Act.Square,
                accum_out=partials[:, 6:7],
            )

            # ship the per-partition partials; the host sums the 128 rows
            nc.sync.dma_start(out=out_h[:], in_=partials[:])

    nc.compile()
    return nc


def get_compiled():
    if "nc" not in _CACHE:
        _CACHE["nc"] = build_bass()
    return _CACHE["nc"]


def _hilo(a, bf16):
    """Lossless-ish bf16 hi/lo split of a float32 array."""
    hi = a.astype(bf16)
    lo = (a - hi.astype(np.float32)).astype(bf16)
    return hi, lo


def make_in_maps(v, v_pred, mask, pred_dw):
    import ml_dtypes

    bf16 = ml_dtypes.bfloat16
    v = np.asarray(v, np.float32)
    v_pred = np.asarray(v_pred, np.float32)
    mask = np.asarray(mask, np.float32)
    pred_dw = np.asarray(pred_dw, np.float32)

    # bf16 hi/lo repacking of the matmul operands, norms included
    wT = (-2.0 * v_pred).transpose(0, 2, 1)           # (16, 3, 2048) f32
    wh, wl = _hilo(wT, bf16)
    yT = v.transpose(0, 2, 1)
    yh, yl = _hilo(yT, bf16)
    x2 = (v_pred * v_pred).sum(-1)[:, None, :]        # (16, 1, 2048)
    x2h, x2l = _hilo(x2, bf16)
    y2 = (v * v).sum(-1)[:, None, :]
    y2h, y2l = _hilo(y2, bf16)
    ones = np.ones((B, 2, P1), dtype=bf16)
    # lhsT rows: [wh x3, wl x3, wh x3, 1, 1, x2h, x2l]
    xprod = np.concatenate([wh, wl, wh, ones, x2h, x2l], axis=1)  # (16,13,2048)
    # rhs rows:  [yh x3, yh x3, yl x3, y2h, y2l, 1, 1]
    yprod = np.concatenate([yh, yh, yl, y2h, y2l, ones], axis=1)

    mask_flat = mask.reshape(B, P2)
    # maskT[b, p, c] = mask_flat[b, c*128 + p]
    maskT = np.ascontiguousarray(
        mask_flat.reshape(B, NC128, 128).transpose(0, 2, 1)
    )
    in_maps = []
    for k in range(NCORES):
        b0 = BPC * k
        dwp = np.concatenate(
            [pred_dw[b0 + i].reshape(128, 48) for i in range(BPC)], axis=1
        )
        in_maps.append({
            "xprod": np.ascontiguousarray(xprod[b0:b0 + BPC]),
            "yprod": np.ascontiguousarray(yprod[b0:b0 + BPC]),
            "maskT": np.ascontiguousarray(maskT[b0:b0 + BPC]),
            "dw": np.ascontiguousarray(dwp),
        })
    return in_maps


def combine_outs(outs):
    """outs: (8, 128, 8) per-core per-partition partials -> (loss, ...)."""
    outs = np.asarray(outs, np.float64).sum(axis=1)
    mcols = [2 * i for i in range(BPC)]
    rcols = [2 * i + 1 for i in range(BPC)]
    msum = outs[:, mcols].sum()
    rsum = outs[:, rcols].sum()
    dsum = outs[:, 6].sum()
    loss = msum / (B * P2) + rsum / (B * P1) + dsum / (B * P1 * D)
    return (np.float32(loss), np.float32(0.0))


def kernel(**inputs):
    from concourse.bass_utils import run_bass_kernel_spmd

    nc = get_compiled()
    in_maps = make_in_maps(
        inputs["v"], inputs["v_pred"], inputs["mask"], inputs["pred_dw"]
    )
    res = run_bass_kernel_spmd(nc, in_maps, core_ids=list(range(NCORES)))
    outs = np.stack([r["out"].reshape(128, 8) for r in res.results])
    return combine_outs(outs)


# revision 40
# speedup vs baseline: 1.0582x; 1.0101x over previous
"""Chamfer-distance loss (CCHLoss) kernel for 8 Trainium2 NeuronCores.

Contract: kernel(**inputs) takes the FULL unsharded inputs
  v:        (16, 2048, 3) f32
  v_pred:   (16, 2048, 3) f32
  mask:     (4, 4, 2, 32, 32) f32
  pred_dw:  (16, 2048, 3) f32
and returns (loss, loss_normals) matching reference().

Strategy: data-parallel over the B=16 batch dim, 2 batches per core.
Per batch the 2048x2048 squared-distance matrix is produced by TensorE
via a K=13 matmul (bf16 hi/lo split of both the -2x*y products and the
|x|^2 / |y|^2 norm rows, all packed on HOST) in 16 PSUM groups of
[128, 2048].  ScalarE evacuates each group to SBUF bf16; VectorE
reduces it with tensor_tensor min trees:
  - per group a fold chain 2048->128 plus one tensor_reduce gives the
    row-min (-> cham_pred),
  - a bf16 min pyramid accumulates the column-min (-> colacc),
  - PE transposes + reduce fold colacc's 128 partitions (-> cham_v),
  - mask-weighted sums reduce everything to per-core scalars.
Host only shards/packs inputs and sums 8 cores' partial sums.
"""

import numpy as np

B, P1, P2, D = 16, 2048, 2048, 3
NCORES = 8
BPC = B // NCORES  # batches per core
NT = P1 // 128     # i-tiles per batch
NJ = P2 // 512     # matmul j-chunks per group
NC128 = P2 // 128  # 128-wide j-chunks (transpose fold)

_CACHE = {}


def build_bass():
    """Build + compile the per-core Bass program (same program all 8 cores)."""
    import concourse.bacc as bacc
    import concourse.tile as tile
    from concourse import mybir
    from concourse.masks import make_identity

    f32 = mybir.dt.float32
    bf16 = mybir.dt.bfloat16
    Alu = mybir.AluOpType
    Act = mybir.ActivationFunctionType
    X = mybir.AxisListType.X

    nc = bacc.Bacc("TRN2", target_bir_lowering=False, debug=False)

    xprod_h = nc.dram_tensor("xprod", (BPC, 13, P1), bf16, kind="ExternalInput")
    yprod_h = nc.dram_tensor("yprod", (BPC, 13, P2), bf16, kind="ExternalInput")
    maskT_h = nc.dram_tensor("maskT", (BPC, 128, NC128), f32, kind="ExternalInput")
    dw_h = nc.dram_tensor("dw", (128, BPC * 48), f32, kind="ExternalInput")
    # per-partition partials go to the host unsummed: dropping the final
    # PE ones-matmul + res copy shortens the tail by ~0.6us
    out_h = nc.dram_tensor("out", (128, 8), f32, kind="ExternalOutput")

    with tile.TileContext(nc) as tc:
        with (
            tc.tile_pool(name="consts", bufs=1) as consts,
            tc.tile_pool(name="opnds", bufs=2) as opnds,
            tc.tile_pool(name="scr", bufs=3) as scr,
            tc.tile_pool(name="small", bufs=4) as small,
            tc.tile_pool(name="ps", bufs=2, space="PSUM") as ps,
        ):
            ident = consts.tile([128, 128], bf16)
            make_identity(nc, ident)
            partials = consts.tile([128, 8], f32)
            nc.vector.memset(partials, 0.0)

            # operand DMAs for BOTH batches up front; the two HWDGE
            # queues split the work.
            e_x = nc.scalar
            e_y = nc.sync
            lhsTs, rhss = [], []
            for b in range(BPC):
                lhsT = opnds.tile([13, P1], bf16)
                rhs = opnds.tile([13, P2], bf16)
                if b == 0:
                    # split batch 0's DMAs so the first matmuls (needing
                    # lhsT[:, :128] and rhs[:, :512]) start ~1us sooner.
                    # rhs's second half goes through the gpsimd SWDGE
                    # queue: both halves transfer CONCURRENTLY instead of
                    # serializing on one HWDGE queue (measured +2us stall).
                    e_y.dma_start(out=lhsT[:, 0:256], in_=xprod_h[b][:, 0:256])
                    e_x.dma_start(out=rhs[:, 0:1024], in_=yprod_h[b][:, 0:1024])
                    nc.gpsimd.dma_start(
                        out=rhs[:, 1024:P2], in_=yprod_h[b][:, 1024:P2]
                    )
                    e_y.dma_start(out=lhsT[:, 256:P1], in_=xprod_h[b][:, 256:P1])
                else:
                    e_y.dma_start(out=lhsT[:], in_=xprod_h[b])
                    e_x.dma_start(out=rhs[:], in_=yprod_h[b])
                lhsTs.append(lhsT)
                rhss.append(rhs)
            mks = []
            for b in range(BPC):
                mk = small.tile([128, NC128], f32, tag="mk", bufs=2)
                nc.sync.dma_start(out=mk[:], in_=maskT_h[b])
                mks.append(mk)

            # tiny warm activation with no data deps: pulls the ~1.3us
            # ACT_TABLE_LOAD to t=0 so the first evacuation isn't delayed
            warm = consts.tile([1, 1], f32)
            nc.vector.memset(warm, 0.0)
            warmo = consts.tile([1, 1], f32)
            nc.scalar.activation(out=warmo[:], in_=warm[:], func=Act.Square)

            # small PE warm-up burst sized to fit INSIDE the ~2.7us input
            # DMA wait: the HAM clock-gate reaches 2.4 GHz sooner without
            # delaying the first real matmul.
            wsrc = consts.tile([13, 256], bf16)
            nc.vector.memset(wsrc, 1.0)
            wps = ps.tile([128, P2], f32, tag="dgrp")
            for _ in range(8):
                nc.tensor.matmul(wps[:, 0:256], wsrc[:, 0:128], wsrc[:])

            # ---------- main distance + min pipeline ----------
            # No PE warm-up burst: while the HAM clock-gate is cold the PE
            # still outruns the ACT evacuation (1.7us vs 2.0us per group),
            # so cold mains cost nothing and start ~4us earlier.
            colaccs, rowparts_l = [], []

            def emit_finals(b):
                """Fold batch b's accumulators to per-batch scalars."""
                colacc = colaccs[b]
                rowparts = rowparts_l[b]
                rowaccs = small.tile([128, NT], f32, bufs=2)
                chamv = small.tile([128, NC128], f32, bufs=2)
                # finish row-min: [128, 16, 128] -> [128, 16] via bf16 TT
                # folds (2x mode) + a short 1x reduce — ~35% faster than a
                # single 1x tensor_reduce over FD=2048.
                ra1 = small.tile([128, NT, 64], bf16, tag="ra1", bufs=1)
                nc.vector.tensor_tensor(
                    out=ra1[:], in0=rowparts[:, :, 0:64],
                    in1=rowparts[:, :, 64:128], op=Alu.min,
                )
                ra2 = small.tile([128, NT, 16], bf16, tag="ra2", bufs=1)
                nc.vector.tensor_tensor(
                    out=ra2[:], in0=ra1[:, :, 0:16], in1=ra1[:, :, 16:32],
                    op=Alu.min,
                )
                nc.vector.tensor_tensor(
                    out=ra2[:], in0=ra2[:], in1=ra1[:, :, 32:48], op=Alu.min,
                )
                nc.vector.tensor_tensor(
                    out=ra2[:], in0=ra2[:], in1=ra1[:, :, 48:64], op=Alu.min,
                )
                nc.vector.tensor_reduce(
                    out=rowaccs[:], in_=ra2[:], axis=X, op=Alu.min
                )
                # fold colacc partitions via PE transpose: all 16 blocks
                # into ONE tile, ONE reduce (saves an op + overhead)
                tp = ps.tile([128, P2], bf16, tag="dgrp")
                for cidx in range(16):
                    nc.tensor.transpose(
                        tp[:, cidx * 128:(cidx + 1) * 128],
                        colacc[:, cidx * 128:(cidx + 1) * 128],
                        ident[:],
                    )
                tpv = tp[:].rearrange("p (a b) -> p a b", b=128)
                nc.vector.tensor_reduce(
                    out=chamv[:], in_=tpv, axis=X, op=Alu.min,
                )
                # per-batch scalars
                prod = small.tile([128, NC128], f32, bufs=2)
                nc.vector.tensor_tensor(
                    out=prod[:], in0=chamv[:], in1=mks[b][:], op=Alu.mult
                )
                nc.vector.tensor_reduce(
                    out=partials[:, 2 * b:2 * b + 1], in_=prod[:], axis=X,
                    op=Alu.add,
                )
                nc.vector.tensor_reduce(
                    out=partials[:, 2 * b + 1:2 * b + 2], in_=rowaccs[:],
                    axis=X, op=Alu.add,
                )
            for b in range(BPC):
                lhsT, rhs = lhsTs[b], rhss[b]
                colacc = opnds.tile([128, P2], bf16)
                rowparts = opnds.tile([128, NT, 128], bf16)
                colaccs.append(colacc)
                rowparts_l.append(rowparts)

                # ramp in with single/pair steps (shortest pipeline fill;
                # a quad's fold must wait for 4 evacuations; pairs also let
                # the slower-per-group DVE fall behind ACT gradually so the
                # first quads never stall), then quad-steps once flowing.
                if b == 0:
                    steps = [[0], [1], [2, 3], [4, 5], [6, 7]] + [
                        list(range(s, s + 4)) for s in range(8, NT, 4)
                    ]
                else:
                    steps = [list(range(s, s + 4)) for s in range(0, NT, 4)]
                first = True
                for si, tiles in enumerate(steps):
                    last_step = b == BPC - 1 and si == len(steps) - 1
                    nu = len(tiles)
                    nb = 3 if nu == 4 else 2
                    s4 = scr.tile([128, nu, P2], bf16, tag=f"s{nu}", bufs=nb)
                    for u, t in enumerate(tiles):
                        g = ps.tile([128, P2], f32, tag="dgrp")
                        lsl = lhsT[:, t * 128:(t + 1) * 128]
                        for c in range(NJ):
                            sl = slice(c * 512, (c + 1) * 512)
                            nc.tensor.matmul(g[:, sl], lsl, rhs[:, sl])
                        # ACT evacuates PSUM -> SBUF bf16 (monolithic:
                        # PSUM dep-tracking is per-tile, so chunked copies
                        # still wait on all 4 matmuls and only add time)
                        nc.scalar.copy(out=s4[:, u, :], in_=g[:])

                    def row_chain():
                        # merged row-min fold chain over the step's groups.
                        # For quads the first level is emitted per-PAIR so
                        # each DVE op waits on 2 evacuations, not 4
                        # (+58cyc, kills the step-transition bubbles).
                        f1 = scr.tile(
                            [128, nu, 1024], bf16, tag=f"f1{nu}", bufs=nb
                        )
                        if nu == 4:
                            nc.vector.tensor_tensor(
                                out=f1[:, 0:2, :], in0=s4[:, 0:2, 0:1024],
                                in1=s4[:, 0:2, 1024:2048], op=Alu.min,
                            )
                            nc.vector.tensor_tensor(
                                out=f1[:, 2:4, :], in0=s4[:, 2:4, 0:1024],
                                in1=s4[:, 2:4, 1024:2048], op=Alu.min,
                            )
                        else:
                            nc.vector.tensor_tensor(
                                out=f1[:], in0=s4[:, :, 0:1024],
                                in1=s4[:, :, 1024:2048], op=Alu.min,
                            )
                        f2 = scr.tile(
                            [128, nu, 512], bf16, tag=f"f2{nu}", bufs=nb
                        )
                        nc.vector.tensor_tensor(
                            out=f2[:], in0=f1[:, :, 0:512],
                            in1=f1[:, :, 512:1024], op=Alu.min,
                        )
                        f3 = scr.tile(
                            [128, nu, 256], bf16, tag=f"f3{nu}", bufs=nb
                        )
                        nc.vector.tensor_tensor(
                            out=f3[:], in0=f2[:, :, 0:256],
                            in1=f2[:, :, 256:512], op=Alu.min,
                        )
                        nc.vector.tensor_tensor(
                            out=rowparts[:, tiles[0]:tiles[0] + nu, :],
                            in0=f3[:, :, 0:128], in1=f3[:, :, 128:256],
                            op=Alu.min,
                        )

                    def col_chain():
                        nonlocal first
                        if nu == 4:
                            # per-pair first level (same reason as f1)
                            m2 = scr.tile([128, 2, P2], bf16, tag="m2")
                            nc.vector.tensor_tensor(
                                out=m2[:, 0:1, :], in0=s4[:, 0:1, :],
                                in1=s4[:, 1:2, :], op=Alu.min,
                            )
                            nc.vector.tensor_tensor(
                                out=m2[:, 1:2, :], in0=s4[:, 2:3, :],
                                in1=s4[:, 3:4, :], op=Alu.min,
                            )
                            if first:
                                # fold straight into colacc: saves the
                                # separate init copy
                                nc.vector.tensor_tensor(
                                    out=colacc[:], in0=m2[:, 0, :],
                                    in1=m2[:, 1, :], op=Alu.min,
                                )
                                first = False
                                return
                            mm = scr.tile([128, P2], bf16, tag="mm")
                            nc.vector.tensor_tensor(
                                out=mm[:], in0=m2[:, 0, :], in1=m2[:, 1, :],
                                op=Alu.min,
                            )
                        elif nu == 2:
                            if first:
                                nc.vector.tensor_tensor(
                                    out=colacc[:], in0=s4[:, 0, :],
                                    in1=s4[:, 1, :], op=Alu.min,
                                )
                                first = False
                                return
                            mm = scr.tile([128, P2], bf16, tag="mm")
                            nc.vector.tensor_tensor(
                                out=mm[:], in0=s4[:, 0, :], in1=s4[:, 1, :],
                                op=Alu.min,
                            )
                        else:  # nu == 1
                            mm = None
                        src = s4[:, 0, :] if mm is None else mm[:]
                        if first:
                            nc.vector.tensor_copy(out=colacc[:], in_=src)
                            first = False
                        else:
                            nc.vector.tensor_tensor(
                                out=colacc[:], in0=colacc[:], in1=src,
                                op=Alu.min,
                            )

                    # on the very last step, finish the col path first so
                    # the final colacc transposes (PE) start ~2us earlier;
                    # the row chain then runs under them.
                    if last_step:
                        col_chain()
                        row_chain()
                    else:
                        row_chain()
                        col_chain()

            # finals after all mains: the PE processes its queue far ahead
            # of the DVE, so transposes emitted any earlier stall the
            # in-order PE queue waiting on colacc (measured: -7us).
            for b in range(BPC):
                emit_finals(b)

            # mean(pred_dw^2) partial: ACT is idle for the last ~20us of
            # the span, so this costs nothing here; emitting it earlier
            # delays the evacuation stream.
            dwt = consts.tile([128, BPC * 48], f32)
            nc.sync.dma_start(out=dwt[:], in_=dw_h[:])
            dwsq = consts.tile([128, BPC * 48], f32)
            nc.scalar.activation(
                out=dwsq[:], in_=dwt[:], func=Act.Square,
                accum_out=partials[:, 6:7],
            )

            # ship the per-partition partials; the host sums the 128 rows
            nc.sync.dma_start(out=out_h[:], in_=partials[:])

    nc.compile()
    return nc


def get_compiled():
    if "nc" not in _CACHE:
        _CACHE["nc"] = build_bass()
    return _CACHE["nc"]


def _hilo(a, bf16):
    """Lossless-ish bf16 hi/lo split of a float32 array."""
    hi = a.astype(bf16)
    lo = (a - hi.astype(np.float32)).astype(bf16)
    return hi, lo


def make_in_maps(v, v_pred, mask, pred_dw):
    import ml_dtypes

    bf16 = ml_dtypes.bfloat16
    v = np.asarray(v, np.float32)
    v_pred = np.asarray(v_pred, np.float32)
    mask = np.asarray(mask, np.float32)
    pred_dw = np.asarray(pred_dw, np.float32)

    # bf16 hi/lo repacking of the matmul operands, norms included
    wT = (-2.0 * v_pred).transpose(0, 2, 1)           # (16, 3, 2048) f32
    wh, wl = _hilo(wT, bf16)
    yT = v.transpose(0, 2, 1)
    yh, yl = _hilo(yT, bf16)
    x2 = (v_pred * v_pred).sum(-1)[:, None, :]        # (16, 1, 2048)
    x2h, x2l = _hilo(x2, bf16)
    y2 = (v * v).sum(-1)[:, None, :]
    y2h, y2l = _hilo(y2, bf16)
    ones = np.ones((B, 2, P1), dtype=bf16)
    # lhsT rows: [wh x3, wl x3, wh x3, 1, 1, x2h, x2l]
    xprod = np.concatenate([wh, wl, wh, ones, x2h, x2l], axis=1)  # (16,13,2048)
    # rhs rows:  [yh x3, yh x3, yl x3, y2h, y2l, 1, 1]
    yprod = np.concatenate([yh, yh, yl, y2h, y2l, ones], axis=1)

    mask_flat = mask.reshape(B, P2)
    # maskT[b, p, c] = mask_flat[b, c*128 + p]
    maskT = np.ascontiguousarray(
        mask_flat.reshape(B, NC128, 128).transpose(0, 2, 1)
    )
    in_maps = []
    for k in range(NCORES):
        b0 = BPC * k
        dwp = np.concatenate(
            [pred_dw[b0 + i].reshape(128, 48) for i in range(BPC)], axis=1
        )
        in_maps.append({
            "xprod": np.ascontiguousarray(xprod[b0:b0 + BPC]),
            "yprod": np.ascontiguousarray(yprod[b0:b0 + BPC]),
            "maskT": np.ascontiguousarray(maskT[b0:b0 + BPC]),
            "dw": np.ascontiguousarray(dwp),
        })
    return in_maps


def combine_outs(outs):
    """outs: (8, 128, 8) per-core per-partition partials -> (loss, ...)."""
    outs = np.asarray(outs, np.float64).sum(axis=1)
    mcols = [2 * i for i in range(BPC)]
    rcols = [2 * i + 1 for i in range(BPC)]
    msum = outs[:, mcols].sum()
    rsum = outs[:, rcols].sum()
    dsum = outs[:, 6].sum()
    loss = msum / (B * P2) + rsum / (B * P1) + dsum / (B * P1 * D)
    return (np.float32(loss), np.float32(0.0))


def kernel(**inputs):
    from concourse.bass_utils import run_bass_kernel_spmd

    nc = get_compiled()
    in_maps = make_in_maps(
        inputs["v"], inputs["v_pred"], inputs["mask"], inputs["pred_dw"]
    )
    res = run_bass_kernel_spmd(nc, in_maps, core_ids=list(range(NCORES)))
    outs = np.stack([r["out"].reshape(128, 8) for r in res.results])
    return combine_outs(outs)
